# revision 28
# baseline (speedup 1.0000x reference)
"""Trainium2 Bass kernel: 3-layer GAT message passing, 8-core SPMD.

Dst-sharded edge phase with per-edge dma_gather (4 SWDGE queues round-robin),
host-precomputed fp8 one-hot scatter/expand matrices, node tables split into
4 chunks per layer (separate DRAM tiles) so chunked AllGathers pipeline with
the edge phase without WAR stalls.
"""

import math
from dataclasses import dataclass, field

import numpy as np
import ml_dtypes

import concourse.bass as bass
import concourse.bacc as bacc
import concourse.mybir as mybir
import concourse.tile as tile
from concourse import library_config

F32 = mybir.dt.float32
BF16 = mybir.dt.bfloat16
I16 = mybir.dt.int16
F8 = mybir.dt.float8e4
AF = mybir.ActivationFunctionType
ALU = mybir.AluOpType
NEG_SLOPE = 0.2

bf = ml_dtypes.bfloat16
f8 = np.dtype(ml_dtypes.float8_e4m3)


@dataclass
class Cfg:
    N: int = 50000
    n_cores: int = 8
    IN: int = 256
    HID: int = 128
    OUT: int = 128
    K_FUSE: int = 8
    n_queues: int = 4
    n_chunks: int = 4

    @property
    def n_loc(self):
        return self.N // self.n_cores

    @property
    def n_win(self):
        return (self.n_loc + 127) // 128

    def win_size(self, t):
        return min(128, self.n_loc - t * 128)

    @property
    def chunk_bounds(self):
        W, K = self.n_win, self.n_chunks
        base, rem = divmod(W, K)
        b = [0]
        for i in range(K):
            b.append(b[-1] + base + (1 if i < rem else 0))
        return b

    def chunk_of_win(self, t):
        b = self.chunk_bounds
        for k in range(self.n_chunks):
            if t < b[k + 1]:
                return k
        raise ValueError(t)

    def chunk_rows(self, k):
        b = self.chunk_bounds
        return min(b[k + 1] * 128, self.n_loc) - b[k] * 128

    @property
    def layers(self):
        hid, out = self.HID, self.OUT
        ls = []
        for (H, C, in_ch) in ((4, hid // 2, hid), (2, out, 2 * hid), (1, out, out)):
            HC = H * C
            row = HC + 2 * H
            row_pad = ((row * 2 + 255) // 256) * 256 // 2
            ls.append(dict(H=H, C=C, in_ch=in_ch, HC=HC, row=row_pad,
                           as_off=HC, ad_off=HC + H))
        return ls


# ---------------------------------------------------------------- host plan

@dataclass
class Plan:
    T: list                  # T[k][t]: gather tiles for (chunk k, window t)
    idx: list                # idx[k][c]: [128, cols_k] int16
    cols: list               # cols[k]
    win_tile_off: list       # tile offset of window t within the one-hot cols
    win_off: list            # win_off[k][t]: idx column offset (tiles*8)
    oT: list = field(default_factory=list)
    oD: list = field(default_factory=list)
    NT: int = 0
    MAXT: int = 0


def plan_edges(edge_index, cfg: Cfg) -> Plan:
    N, C, K = cfg.N, cfg.n_cores, cfg.n_chunks
    n_loc = cfg.n_loc
    W = cfg.n_win
    bounds = cfg.chunk_bounds
    rows = [cfg.chunk_rows(k) for k in range(K)]
    starts = [bounds[k] * 128 for k in range(K)]

    src = np.asarray(edge_index[0], np.int64)   # self loops handled by a
    dst = np.asarray(edge_index[1], np.int64)   # direct per-window load
    core = dst // n_loc
    win = (dst % n_loc) // 128
    dloc = (dst % n_loc) % 128
    s_core = src // n_loc
    s_loc = src % n_loc
    s_chunk = np.zeros_like(s_loc)
    idx_all = np.zeros_like(s_loc)
    for k in range(K):
        m = (s_loc >= starts[k]) & (s_loc < starts[k] + rows[k])
        s_chunk[m] = k
        idx_all[m] = s_core[m] * rows[k] + (s_loc[m] - starts[k])

    buckets = {}
    for c in range(C):
        m_c = core == c
        for t in range(W):
            m_t = m_c & (win == t)
            for k in range(K):
                m = m_t & (s_chunk == k)
                si, dl = idx_all[m], dloc[m]
                o = np.argsort(si, kind="stable")
                buckets[(c, t, k)] = (si[o], dl[o])

    T = [[max(math.ceil(len(buckets[(c, t, k)][0]) / 128) for c in range(C))
          for t in range(W)] for k in range(K)]

    NT = sum(sum(Tk) for Tk in T) + W  # +1 self tile per window
    win_tile_off = []
    win_off = [[0] * W for _ in range(K)]
    off = 0
    okk = [0] * K
    for t in range(W):
        win_tile_off.append(off)
        for k in range(K):
            win_off[k][t] = okk[k]
            okk[k] += T[k][t] * 8
            off += T[k][t]
        off += 1  # self tile
    cols = [max(okk[k], 8) for k in range(K)]
    MAXT = max(sum(T[k][t] for k in range(K)) for t in range(W)) + 1

    p = Plan(T=T, idx=[[] for _ in range(K)], cols=cols,
             win_tile_off=win_tile_off, win_off=win_off, NT=NT, MAXT=MAXT)

    ar = np.arange(128, dtype=np.float32)
    for c in range(C):
        dcol = np.full((128, NT), -1.0, np.float32)
        idx_arrs = [np.zeros((128, cols[k]), np.int16) for k in range(K)]
        for t in range(W):
            tb = win_tile_off[t]
            for k in range(K):
                Tn = T[k][t]
                if Tn:
                    s, dl = buckets[(c, t, k)]
                    n = Tn * 128
                    si = np.zeros(n, np.int64)
                    si[:len(s)] = s
                    dli = np.full(n, -1.0, np.float32)
                    dli[:len(dl)] = dl
                    blk = si.astype(np.int16).reshape(Tn * 8, 16).T
                    idx_arrs[k][:, win_off[k][t]:win_off[k][t] + Tn * 8] = \
                        np.tile(blk, (8, 1))
                    dcol[:, tb:tb + Tn] = dli.reshape(Tn, 128).T
                tb += Tn
            nn_w = cfg.win_size(t)
            selfcol = np.full(128, -1.0, np.float32)
            selfcol[:nn_w] = np.arange(nn_w)
            dcol[:, tb] = selfcol
        for k in range(K):
            p.idx[k].append(idx_arrs[k])
        eq = (dcol[:, :, None] == ar[None, None, :])
        p.oT.append(np.ascontiguousarray(
            eq.reshape(128, NT * 128).astype(np.float32)).astype(f8))
        p.oD.append(np.ascontiguousarray(
            eq.transpose(2, 1, 0).reshape(128, NT * 128)
            .astype(np.float32)).astype(f8))
    return p


# ------------------------------------------------------------- host weights

def prep_weights(inp, cfg: Cfg):
    out = {}
    for li, (wk, ak, dk) in enumerate(
            (("g1_W", "g1_as", "g1_ad"), ("g2_W", "g2_as", "g2_ad"),
             ("g3_W", "g3_as", "g3_ad"))):
        L = cfg.layers[li]
        Wm = np.asarray(inp[wk], np.float32)
        a_s = np.asarray(inp[ak], np.float32)
        a_d = np.asarray(inp[dk], np.float32)
        H, Cc = L["H"], L["C"]
        U_s = np.stack([Wm[:, h * Cc:(h + 1) * Cc] @ a_s[h] for h in range(H)], 1)
        U_d = np.stack([Wm[:, h * Cc:(h + 1) * Cc] @ a_d[h] for h in range(H)], 1)
        out[f"WG{li+1}"] = np.concatenate([Wm, U_s, U_d], 1).astype(bf)
        out[f"bG{li+1}"] = np.ascontiguousarray(np.broadcast_to(
            np.asarray(inp[f"g{li+1}_b"], np.float32)[None, :],
            (128, len(inp[f"g{li+1}_b"]))))
    out["Wm1"] = np.asarray(inp["W1"], np.float32).astype(bf)
    out["Wm2"] = np.asarray(inp["W2"], np.float32).astype(bf)
    out["b1c"] = np.ascontiguousarray(np.asarray(inp["b1"], np.float32)[:, None])
    out["b2c"] = np.ascontiguousarray(np.asarray(inp["b2"], np.float32)[:, None])
    out["ident"] = np.eye(128, dtype=np.float32).astype(bf)
    return out


# ---------------------------------------------------------------- builder

def build(nc, cfg: Cfg, p: Plan):
    W = cfg.n_win
    K = cfg.n_chunks
    n_loc = cfg.n_loc
    Ls = cfg.layers
    bounds = cfg.chunk_bounds
    rows = [cfg.chunk_rows(k) for k in range(K)]
    starts = [bounds[k] * 128 for k in range(K)]
    MAXT = p.MAXT
    MAXHC = max(L["HC"] for L in Ls)
    MAXH = max(L["H"] for L in Ls)

    def din(name, shape, dt):
        return nc.dram_tensor(name, list(shape), dt, kind="ExternalInput")

    xT = din("xT", (cfg.IN, n_loc), BF16)
    Wm1 = din("Wm1", (cfg.IN, cfg.HID), BF16)
    Wm2 = din("Wm2", (cfg.HID, cfg.HID), BF16)
    b1c = din("b1c", (cfg.HID, 1), F32)
    b2c = din("b2c", (cfg.HID, 1), F32)
    WG = [din(f"WG{i+1}", (Ls[i]["in_ch"], Ls[i]["HC"] + 2 * Ls[i]["H"]), BF16)
          for i in range(3)]
    bG = [din(f"bG{i+1}", (128, Ls[i]["HC"] if i == 0 else Ls[i]["C"]), F32)
          for i in range(3)]
    ident_d = din("ident", (128, 128), BF16)
    idx_d = [din(f"idx{k}", (128, p.cols[k]), I16) for k in range(K)]
    oT_d = din("oT", (128, p.NT * 128), F8)
    oD_d = din("oD", (128, p.NT * 128), F8)
    out_d = nc.dram_tensor("out", [n_loc, cfg.OUT], F32, kind="ExternalOutput")

    rep = [list(range(cfg.n_cores))]

    with tile.TileContext(nc) as tc:
        with (
            tc.tile_pool(name="const", bufs=1) as cpool,
            tc.tile_pool(name="gat", bufs=3) as gpool,
            tc.tile_pool(name="oh", bufs=3) as opool,
            tc.tile_pool(name="msg", bufs=2) as mpool,
            tc.tile_pool(name="small", bufs=4) as spool,
            tc.tile_pool(name="nodes", bufs=3) as npool,
            tc.tile_pool(name="psA", bufs=2, space="PSUM") as psA,
            tc.tile_pool(name="psB", bufs=2, space="PSUM") as psB,
            tc.tile_pool(name="psC", bufs=2, space="PSUM") as psC,
            tc.tile_pool(name="dram", bufs=1, space="DRAM") as dpool,
        ):
            nc.gpsimd.load_library(library_config.mlp)
            gq = [0]  # round-robin SWDGE queue counter for gathers

            def load_const(handle, shape, dtp, tag):
                t = cpool.tile(list(shape), dtp, tag=tag, name=tag)
                nc.sync.dma_start(t[:], handle[:])
                return t

            ident = load_const(ident_d, (128, 128), BF16, "ident")
            idx_sb = [load_const(idx_d[k], (128, p.cols[k]), I16, f"idx{k}")
                      for k in range(K)]
            wg_sb = []
            for i in range(3):
                L = Ls[i]
                kch = L["in_ch"] // 128
                t = cpool.tile([128, kch, L["HC"] + 2 * L["H"]], BF16,
                               tag=f"wg{i}", name=f"wg{i}")
                for k in range(kch):
                    nc.sync.dma_start(t[:, k, :], WG[i][k * 128:(k + 1) * 128, :])
                wg_sb.append(t)
            bg_sb = [load_const(bG[i], (128, Ls[i]["HC"] if i == 0 else Ls[i]["C"]),
                                F32, f"bg{i}") for i in range(3)]
            wm1 = cpool.tile([128, 2, cfg.HID], BF16, tag="wm1")
            for k in range(2):
                nc.sync.dma_start(wm1[:, k, :], Wm1[k * 128:(k + 1) * 128, :])
            wm2 = load_const(Wm2, (cfg.HID, cfg.HID), BF16, "wm2")
            b1s = load_const(b1c, (cfg.HID, 1), F32, "b1s")
            b2s = load_const(b2c, (cfg.HID, 1), F32, "b2s")

            piece = [dpool.tile([cfg.HID, n_loc], BF16, tag="piece0",
                                name="piece0"),
                     dpool.tile([Ls[0]["HC"], n_loc], BF16, tag="piece1",
                                name="piece1"),
                     dpool.tile([Ls[1]["C"], n_loc], BF16, tag="piece2",
                                name="piece2")]
            tbl_in = [[dpool.tile([rows[k], Ls[i]["row"]], BF16,
                                  tag=f"tin{i}_{k}", name=f"tin{i}_{k}")
                       for k in range(K)] for i in range(3)]
            tbl_ag = [[dpool.tile([rows[k] * cfg.n_cores, Ls[i]["row"]], BF16,
                                  tag=f"tag{i}_{k}", name=f"tag{i}_{k}")
                       for k in range(K)] for i in range(3)]

            # ================= MLP (node-sharded) =================
            CH = 512
            nch = math.ceil(n_loc / CH)
            for j in range(nch):
                n0 = j * CH
                nn = min(CH, n_loc - n0)
                xt = npool.tile([128, 2, CH], BF16, tag="xt")
                for k in range(2):
                    nc.sync.dma_start(xt[:, k, :nn],
                                      xT[k * 128:(k + 1) * 128, n0:n0 + nn])
                ps = psC.tile([128, CH], F32, tag="mm")
                for k in range(2):
                    nc.tensor.matmul(ps[:, :nn], wm1[:, k, :], xt[:, k, :nn],
                                     start=(k == 0), stop=(k == 1))
                h1 = npool.tile([128, CH], BF16, tag="h1")
                nc.scalar.activation(h1[:, :nn], ps[:, :nn], AF.Relu,
                                     bias=b1s[:, 0:1])
                ps2 = psC.tile([128, CH], F32, tag="mm")
                nc.tensor.matmul(ps2[:, :nn], wm2[:, :], h1[:, :nn],
                                 start=True, stop=True)
                h2 = npool.tile([128, CH], BF16, tag="h2")
                nc.scalar.activation(h2[:, :nn], ps2[:, :nn], AF.Relu,
                                     bias=b2s[:, 0:1])
                nc.sync.dma_start(piece[0][:, n0:n0 + nn], h2[:, :nn])

            ad_tiles = {}

            def node_chunk(li, j):
                L = Ls[li]
                kch = L["in_ch"] // 128
                NCOL = L["HC"] + 2 * L["H"]
                ROW = L["row"]
                n0 = j * 128
                nn = cfg.win_size(j)
                kc = cfg.chunk_of_win(j)
                lh = npool.tile([128, kch, 128], BF16, tag="lh", name="lh")
                for k in range(kch):
                    nc.sync.dma_start(
                        lh[:, k, :nn],
                        piece[li][k * 128:(k + 1) * 128, n0:n0 + nn])
                ps = psC.tile([128, NCOL], F32, tag="mm", name="psn")
                for k in range(kch):
                    nc.tensor.matmul(ps[:nn, :], lh[:, k, :nn],
                                     wg_sb[li][:, k, :],
                                     start=(k == 0), stop=(k == kch - 1))
                tb = npool.tile([128, ROW], BF16, tag="tb", name="tb")
                nc.scalar.activation(tb[:nn, :NCOL], ps[:nn, :], AF.Copy)
                m0 = n0 - starts[kc]
                nc.sync.dma_start(
                    tbl_in[li][kc][m0:m0 + nn, :NCOL], tb[:nn, :NCOL])

            def emit_ag(li, k):
                nc.gpsimd.collective_compute(
                    "AllGather", ALU.bypass, replica_groups=rep,
                    ins=[tbl_in[li][k][:, :]], outs=[tbl_ag[li][k][:, :]])

            def emit_ad(li):
                L = Ls[li]
                H = L["H"]
                ad_all = spool.tile([128, W, MAXH], BF16, tag="ad_all",
                                    name="ad_all", bufs=2)
                ad_tiles[li] = ad_all
                nc.vector.memset(ad_all[:], 0.0)
                for k in range(K):
                    full_w = rows[k] // 128
                    w0 = bounds[k]
                    if full_w:
                        ad_f = tbl_in[li][k][:full_w * 128,
                                             L["ad_off"]:L["ad_off"] + H]
                        nc.sync.dma_start(
                            ad_all[:, w0:w0 + full_w, :H],
                            ad_f.rearrange("(w q) h -> q w h", q=128))
                    if rows[k] % 128:
                        rem = rows[k] - full_w * 128
                        nc.sync.dma_start(
                            ad_all[:rem, w0 + full_w, :H],
                            tbl_in[li][k][full_w * 128:,
                                          L["ad_off"]:L["ad_off"] + H])

            for j in range(W):
                node_chunk(0, j)
                for k in range(K):
                    if j == bounds[k + 1] - 1:
                        emit_ag(0, k)
            emit_ad(0)

            # ================= layers =================
            for li in range(3):
                L = Ls[li]
                H, Cc, HC, ROW = L["H"], L["C"], L["HC"], L["row"]
                NCOL = HC + 2 * H
                ad_all = ad_tiles[li]

                for t in range(W):
                    Ts = [p.T[k][t] for k in range(K)]
                    T = sum(Ts) + 1
                    nn = cfg.win_size(t)
                    to = p.win_tile_off[t]
                    kc = cfg.chunk_of_win(t)
                    g = gpool.tile([128, T, ROW], BF16, tag="g")
                    # self-loop rows: direct sequential load, no gather
                    m0 = t * 128 - starts[kc]
                    nc.sync.dma_start(
                        g[:nn, T - 1, :NCOL],
                        tbl_in[li][kc][m0:m0 + nn, :NCOL])
                    GMAX = 8  # tiles per dma_gather (>1024 idxs crashes HW)
                    tbase = 0
                    for k in range(K):
                        for q0 in range(0, Ts[k], GMAX):
                            q = min(GMAX, Ts[k] - q0)
                            nc.gpsimd.dma_gather(
                                g[:, tbase + q0:tbase + q0 + q, :],
                                tbl_ag[li][k][:, :],
                                idx_sb[k][:, p.win_off[k][t] + q0 * 8:
                                          p.win_off[k][t] + (q0 + q) * 8],
                                q * 128, q * 128, ROW,
                                queue_num=gq[0] % cfg.n_queues)
                            gq[0] += 1
                        tbase += Ts[k]

                    oT = opool.tile([128, T, 128], F8, tag="oT")
                    oD = opool.tile([128, T, 128], F8, tag="oD")
                    nc.sync.dma_start(
                        oT.rearrange("p a b -> p (a b)"),
                        oT_d[:, to * 128:(to + T) * 128])
                    nc.sync.dma_start(
                        oD.rearrange("p a b -> p (a b)"),
                        oD_d[:, to * 128:(to + T) * 128])

                    ps_ad = psB.tile([128, MAXT * MAXH], F32, tag="ps_ad")
                    for i in range(T):
                        nc.tensor.matmul(ps_ad[:, i * H:(i + 1) * H],
                                         oD[:, i, :], ad_all[:, t, :H],
                                         start=True, stop=True)
                    e_sb = spool.tile([128, MAXT * MAXH], F32, tag="e_sb")
                    nc.vector.tensor_tensor(
                        e_sb[:, :T * H], ps_ad[:, :T * H],
                        g[:, 0:T, L["as_off"]:L["as_off"] + H],
                        ALU.add)
                    ex1 = spool.tile([128, MAXT * MAXH], F32, tag="ex1")
                    nc.scalar.activation(ex1[:, :T * H], e_sb[:, :T * H], AF.Exp)
                    ex2 = spool.tile([128, MAXT * MAXH], F32, tag="ex2")
                    nc.scalar.activation(ex2[:, :T * H], e_sb[:, :T * H], AF.Exp,
                                         scale=NEG_SLOPE)
                    msg = mpool.tile([128, T, HC + H], BF16, tag="msg")
                    nc.vector.tensor_tensor(
                        msg[:, 0:T, HC:HC + H],
                        ex1[:, :T * H], ex2[:, :T * H], ALU.max)
                    for k0 in range(0, T, cfg.K_FUSE):
                        Kf = min(cfg.K_FUSE, T - k0)
                        nc.vector.tensor_tensor(
                            msg[:, k0:k0 + Kf, 0:HC],
                            g[:, k0:k0 + Kf, 0:HC],
                            msg[:, k0:k0 + Kf, HC:HC + H]
                                .unsqueeze(3).broadcast_to([128, Kf, H, Cc]),
                            ALU.mult)
                    ps_w = psA.tile([128, HC + H], F32, tag="ps_w")
                    for i in range(T):
                        nc.tensor.matmul(ps_w[:, :], oT[:, i, :],
                                         msg[:, i, :],
                                         start=(i == 0), stop=(i == T - 1))
                    rcp = spool.tile([128, MAXH], F32, tag="rcp")
                    nc.vector.reciprocal(rcp[:, :H], ps_w[:, HC:HC + H])
                    if li == 1:
                        nc.scalar.activation(rcp[:, :H], rcp[:, :H], AF.Copy,
                                             scale=0.5)
                    y = spool.tile([128, MAXHC], F32, tag="y")
                    nc.vector.tensor_tensor(
                        y[:, :HC], ps_w[:, :HC],
                        rcp[:, :H].unsqueeze(2).broadcast_to([128, H, Cc]),
                        ALU.mult)
                    if li == 1:
                        nc.vector.tensor_tensor(y[:, :Cc], y[:, :Cc],
                                                y[:, Cc:2 * Cc], ALU.add)
                        ycols = Cc
                    else:
                        ycols = HC
                    nc.vector.tensor_tensor(
                        y[:, :ycols], y[:, :ycols],
                        bg_sb[li][:, :ycols], ALU.add)
                    if li < 2:
                        e1 = spool.tile([128, MAXHC], F32, tag="elu1")
                        nc.scalar.activation(e1[:, :ycols], y[:, :ycols], AF.Exp)
                        nc.scalar.activation(e1[:, :ycols], e1[:, :ycols],
                                             AF.Relu, scale=-1.0, bias=1.0)
                        nc.scalar.activation(y[:, :ycols], y[:, :ycols], AF.Relu)
                        yb = spool.tile([128, MAXHC], BF16, tag="yb")
                        nc.vector.tensor_tensor(yb[:, :ycols], y[:, :ycols],
                                                e1[:, :ycols], ALU.subtract)
                        for k in range(ycols // 128):
                            pt = psC.tile([128, 128], BF16, tag="mm")
                            nc.tensor.transpose(pt[:, :],
                                                yb[:, k * 128:(k + 1) * 128],
                                                ident[:, :])
                            pts = spool.tile([128, 128], BF16, tag="pts")
                            nc.scalar.activation(pts[:, :], pt[:, :], AF.Copy)
                            nc.sync.dma_start(
                                piece[li + 1][k * 128:(k + 1) * 128,
                                              t * 128:t * 128 + nn],
                                pts[:, :nn])
                    else:
                        nc.sync.dma_start(out_d[t * 128:t * 128 + nn, :],
                                          y[:nn, :ycols])

                    if li < 2:
                        node_chunk(li + 1, t)
                        for k in range(K):
                            if t == bounds[k + 1] - 1:
                                emit_ag(li + 1, k)
                        if t == W - 1:
                            emit_ad(li + 1)
    return nc, out_d


# ---------------------------------------------------------------- runner

def make_inmaps(inputs, cfg: Cfg, p: Plan):
    wts = prep_weights(inputs, cfg)
    x = np.asarray(inputs["x"], np.float32)
    xT = np.ascontiguousarray(x.T).astype(bf)
    n_loc = cfg.n_loc
    in_maps = []
    for c in range(cfg.n_cores):
        m = dict(wts)
        m["xT"] = np.ascontiguousarray(xT[:, c * n_loc:(c + 1) * n_loc])
        for k in range(cfg.n_chunks):
            m[f"idx{k}"] = p.idx[k][c]
        m["oT"] = p.oT[c]
        m["oD"] = p.oD[c]
        in_maps.append(m)
    return in_maps


def build_program(cfg: Cfg, p: Plan, debug=False):
    nc = bacc.Bacc("TRN2", target_bir_lowering=False, debug=debug,
                   num_devices=cfg.n_cores, num_swdge_queues=cfg.n_queues)
    build(nc, cfg, p)
    nc.compile()
    return nc


# ------------------------------------------------------------- entry point

_CACHE = {}


def kernel(**inputs):
    import numpy as _np
    from concourse.bass_utils import run_bass_kernel_spmd

    cfg = Cfg()
    ei = _np.asarray(inputs["edge_index"])
    key = hash(ei.tobytes())
    if key not in _CACHE:
        p = plan_edges(ei, cfg)
        nc = build_program(cfg, p, debug=False)
        _CACHE[key] = (p, nc)
    p, nc = _CACHE[key]
    in_maps = make_inmaps(inputs, cfg, p)
    res = run_bass_kernel_spmd(nc, in_maps, list(range(cfg.n_cores)))
    out = _np.concatenate([res.results[c]["out"] for c in range(cfg.n_cores)], 0)
    return out.astype(_np.float32)
